# revision 38
# baseline (speedup 1.0000x reference)
"""Trainium2 Bass kernel: batched cross-attention (nn_AttentionTrain).

Per batch element b (one NeuronCore each, data parallel over B=8):
    S = dec @ enc^T            [2048, 2048]
    P = softmax(S, axis=-1)
    C = P @ enc                [2048, 1024]
    out = concat([dec, C], -1) [2048, 2048]

Layout strategy per core:
  - out[:, :H] = dec written from SBUF (f32r tiles hold unmodified f32 bits)
  - matmul1 (scores) in float32r: 1 cyc/row, near-f32 precision; chunks 1-3
    k-outer so the stationary dec^T[k] is reused across 3 matmuls; chunk 0
    prefetched into the softmax wait of the previous m-tile
  - softmax along the free dim: DVE reduce_max, ACT fused exp(S - max) with
    row-sum accumulation, normalization deferred to after matmul2
  - matmul2 (context) in bf16: explicit weight loads are prefetched by the
    PE reorder window (and FWL), so per-matmul weight switches are ~free;
    precision impact is limited to the context average (~2e-3)
  - enc loaded once: staged chunks feed PE transposes (-> enc^T f32r) and a
    DVE cast (-> enc bf16 resident for matmul2)
  - all transposes via PE in f32r, grouped 4 per PSUM bank + one DVE copy
"""

import numpy as np

import concourse.bass as bass
import concourse.mybir as mybir
import concourse.tile as tile
from concourse import bacc
from concourse.bass import ds, ts
from concourse.bass_utils import run_bass_kernel_spmd
from concourse.masks import make_identity

B, S, H = 8, 2048, 1024
P = 128
NT = S // P  # 16 sequence tiles
KH = H // P  # 8 hidden k-tiles
NS = S // 512  # 4 chunks of the scores free dim
FP32 = mybir.dt.float32
F32R = mybir.dt.float32r
BF16 = mybir.dt.bfloat16

MM_DT = F32R  # matmul1 / transpose dtype
MM2_DT = BF16  # matmul2 dtype


def _build(repeat=1):
    nc = bacc.Bacc("TRN2", target_bir_lowering=False, debug=False)
    enc_d = nc.dram_tensor("enc_output", [S, H], FP32, kind="ExternalInput").ap()
    dec_d = nc.dram_tensor("dec_output", [S, H], FP32, kind="ExternalInput").ap()
    out_d = nc.dram_tensor("out", [S, 2 * H], FP32, kind="ExternalOutput").ap()

    # enc as 4 chunks of 4 sequence tiles: [q, p, j, h]
    enc_q = enc_d.rearrange("(q j p) h -> q p j h", p=P, j=4)
    dec_r = dec_d.rearrange("(t p) h -> t p h", p=P)
    out_r = out_d.rearrange("(t p) c -> t p c", p=P)

    AF = mybir.ActivationFunctionType
    AX = mybir.AxisListType
    OP = mybir.AluOpType

    def _in(ap):
        return ap.bitcast(MM_DT)

    with tile.TileContext(nc) as tc:
        with (
            tc.tile_pool(name="const", bufs=1) as const_pool,
            tc.tile_pool(name="encb", bufs=1) as encb_pool,
            tc.tile_pool(name="estg", bufs=3) as stg_pool,
            tc.tile_pool(name="enct", bufs=1) as encT_pool,
            tc.tile_pool(name="decs", bufs=2) as dec_pool,
            tc.tile_pool(name="dect", bufs=2) as decT_pool,
            tc.tile_pool(name="probs", bufs=2) as p_pool,
            tc.tile_pool(name="pt", bufs=2) as pT_pool,
            tc.tile_pool(name="couts", bufs=2) as c_pool,
            tc.tile_pool(name="stats", bufs=3) as st_pool,
            tc.tile_pool(name="psS", bufs=5, space="PSUM") as psS,
            tc.tile_pool(name="psT", bufs=2, space="PSUM") as psT,
            tc.tile_pool(name="psC", bufs=1, space="PSUM") as psC,
        ):
          for _rep in range(repeat):
            ident32 = const_pool.tile([P, P], FP32, name="ident32", tag="ident32")
            make_identity(nc, ident32)
            ident = const_pool.tile([P, P], MM_DT, name="identr", tag="identr")
            nc.vector.tensor_copy(ident, ident32)
            ident_b = const_pool.tile([P, P], MM2_DT, name="identb", tag="identb")
            nc.vector.tensor_copy(ident_b, ident32)

            def build_transposed(dst_groups, srcs, tag, dt=MM_DT):
                """4 PE transposes into one PSUM bank + one DVE copy out."""
                idn = ident if dt == MM_DT else ident_b
                for dst, src4 in zip(dst_groups, srcs):
                    tp = psT.tile([P, 4, P], dt, name=tag, tag="tps")
                    for j, s in enumerate(src4):
                        nc.tensor.transpose(tp[:, j, :], s, idn)
                    nc.vector.tensor_copy(dst, tp)

            dec_tiles = {}

            def load_dec(m):
                d_t = dec_pool.tile([P, H], MM_DT, name="dec_t", tag="dec_t")
                nc.sync.dma_start(out=d_t, in_=_in(dec_r[m]))
                dec_tiles[m] = d_t
                return d_t

            def build_decT(d_t):
                dT = decT_pool.tile([P, KH, P], MM_DT, name="decT_m", tag="decT_m")
                build_transposed(
                    [dT[:, ds(4 * g, 4), :] for g in range(2)],
                    [[d_t[:, ts(4 * g + j, P)] for j in range(4)] for g in range(2)],
                    "tpd",
                )
                return dT

            dec_t = load_dec(0)

            # enc: staged f32r chunks feed (a) PE transposes -> encT f32r and
            # (b) a DVE cast -> resident bf16 tiles for matmul2
            encT = [
                encT_pool.tile([P, S], MM_DT, name=f"enct{k}", tag=f"enct{k}")
                for k in range(KH)
            ]
            enc_b = []
            decT = {0: None}
            first_decT_built = False
            for q in range(4):
                e_q = stg_pool.tile([P, 4, H], MM_DT, name="estg", tag="estg")
                nc.sync.dma_start(out=e_q, in_=_in(enc_q[q]))
                e_b = encb_pool.tile([P, 4, H], MM2_DT, name=f"encb{q}", tag=f"encb{q}")
                enc_b.append((e_q, e_b))
            # build dec^T(0) between the enc DMAs and the transposes
            decT[0] = build_decT(dec_t)

            def enc_btile(t):
                return enc_b[t // 4][1][:, t % 4, :]

            # software pipeline over m-tiles
            st = {}

            def _st(m):
                if m not in st:
                    st[m] = {
                        "s": {},
                        "mx4": st_pool.tile([P, NS], FP32, name="mx4", tag="mx4"),
                    }
                return st[m]

            def emit_mm1_single_chunk(m, n):
                s_n = psS.tile([P, 512], FP32, name="s_n", tag="s_n")
                for k in range(KH):
                    nc.tensor.matmul(
                        s_n,
                        lhsT=decT[m][:, k, :],
                        rhs=encT[k][:, ds(n * 512, 512)],
                        start=(k == 0),
                        stop=(k == KH - 1),
                    )
                nc.vector.tensor_reduce(
                    _st(m)["mx4"][:, ds(n, 1)], s_n, axis=AX.X, op=OP.max
                )
                _st(m)["s"][n] = s_n

            def emit_mm1_chunk0(m):
                emit_mm1_single_chunk(m, 0)

            # setup: transpose each enc quarter as it arrives, then run
            # m=0's score chunk for that quarter immediately
            for q in range(4):
                e_q, e_b = enc_b[q]
                build_transposed(
                    [encT[k][:, ds(512 * q, 512)] for k in range(KH)],
                    [[e_q[:, j, ts(k, P)] for j in range(4)] for k in range(KH)],
                    "tpe",
                )
                nc.vector.tensor_copy(e_b, e_q)
                emit_mm1_single_chunk(0, q)

            def emit_mm1_rest_matmuls(m, filler=None):
                # chunks 1..3 k-outer: stationary decT[k] reused 3x.
                # `filler(k)` lets PE work for m+1 (dec^T transposes) be
                # interleaved mid-stream so its DVE copies finish before the
                # chunk-0 prefetch needs them.
                chunks = {
                    n: psS.tile([P, 512], FP32, name="s_n", tag="s_n")
                    for n in range(1, NS)
                }
                for k in range(KH):
                    for n in range(1, NS):
                        nc.tensor.matmul(
                            chunks[n],
                            lhsT=decT[m][:, k, :],
                            rhs=encT[k][:, ds(n * 512, 512)],
                            start=(k == 0),
                            stop=(k == KH - 1),
                        )
                    if filler is not None:
                        filler(k)
                for n in range(1, NS):
                    _st(m)["s"][n] = chunks[n]

            def emit_mm1_rest_maxes(m):
                for n in range(1, NS):
                    nc.vector.tensor_reduce(
                        _st(m)["mx4"][:, ds(n, 1)], _st(m)["s"][n],
                        axis=AX.X, op=OP.max,
                    )

            for m in range(NT):
                if m == 0:
                    # m=0 chunks/maxes were emitted during setup
                    next_dec = load_dec(1)
                    decT[1] = build_decT(next_dec)
                else:
                    if m + 1 < NT:
                        next_dec = load_dec(m + 1)
                        dT_next = decT_pool.tile(
                            [P, KH, P], MM_DT, name="decT_m", tag="decT_m"
                        )
                        decT[m + 1] = dT_next

                        def _filler(k, d_t=next_dec, dT=dT_next):
                            # two transpose groups, after k=2 and k=4
                            if k in (2, 4):
                                g = 0 if k == 2 else 1
                                build_transposed(
                                    [dT[:, ds(4 * g, 4), :]],
                                    [[d_t[:, ts(4 * g + j, P)] for j in range(4)]],
                                    "tpd",
                                )
                    else:
                        _filler = None

                    emit_mm1_rest_matmuls(m, _filler)
                    emit_mm1_rest_maxes(m)
                neg_mx = st_pool.tile([P, 1], FP32, name="neg_mx", tag="neg_mx")
                nc.vector.tensor_reduce(
                    neg_mx, st[m]["mx4"], axis=AX.X, op=OP.max, negate=True
                )

                # fused exp(S - max) with per-chunk row sums, PSUM -> SBUF.
                # bf16 output: same rounding that the P^T copy applied anyway
                p_t = p_pool.tile([P, S], MM2_DT, name="p_t", tag="p_t")
                sums = st_pool.tile([P, NS], FP32, name="sums", tag="sums")
                for n in range(NS):
                    nc.scalar.activation(
                        p_t[:, ds(n * 512, 512)],
                        st[m]["s"][n],
                        AF.Exp,
                        bias=neg_mx,
                        scale=1.0,
                        accum_out=sums[:, ds(n, 1)],
                    )
                del st[m]
                tot = st_pool.tile([P, 1], FP32, name="tot", tag="tot")
                nc.vector.tensor_reduce(tot, sums, axis=AX.X, op=OP.add)
                rsum = st_pool.tile([P, 1], FP32, name="rsum", tag="rsum")
                nc.vector.reciprocal(rsum, tot)

                if m + 1 < NT:
                    emit_mm1_chunk0(m + 1)

                # pass-through out[:, :H] = dec from SBUF (bit exact); early
                # emission keeps the dec slot rotation ahead of the loads
                d_t = dec_tiles.pop(m)
                nc.sync.dma_start(out=out_r[m][:, ds(0, H)], in_=d_t.bitcast(FP32))

                # P^T tiles [e_part, d] in bf16
                pT_m = pT_pool.tile([P, NT, P], MM2_DT, name="pT_m", tag="pT_m")
                build_transposed(
                    [pT_m[:, ds(4 * g, 4), :] for g in range(4)],
                    [
                        [p_t[:, ts(4 * g + j, P)] for j in range(4)]
                        for g in range(4)
                    ],
                    "tpp",
                    dt=MM2_DT,
                )

                # matmul2 (bf16): C = P @ enc, normalized by 1/rowsum
                c_sb = c_pool.tile([P, H], FP32, name="c_sb", tag="c_sb", bufs=1)
                for h2 in range(2):
                    c_ps = psC.tile([P, 512], FP32, name="c_ps", tag="c_ps")
                    for t in range(NT):
                        nc.tensor.matmul(
                            c_ps,
                            lhsT=pT_m[:, t, :],
                            rhs=enc_btile(t)[:, ds(h2 * 512, 512)],
                            start=(t == 0),
                            stop=(t == NT - 1),
                        )
                    nc.vector.tensor_scalar_mul(
                        c_sb[:, ds(h2 * 512, 512)], c_ps, rsum
                    )

                nc.sync.dma_start(out=out_r[m][:, ds(H, H)], in_=c_sb)
                decT.pop(m, None)

    nc.compile()
    return nc


_nc_cache = {}


def _get_nc(repeat=1):
    if repeat not in _nc_cache:
        _nc_cache[repeat] = _build(repeat)
    return _nc_cache[repeat]


def run(enc_output, dec_output, trace=False):
    nc = _get_nc()
    enc = np.ascontiguousarray(np.asarray(enc_output, dtype=np.float32))
    dec = np.ascontiguousarray(np.asarray(dec_output, dtype=np.float32))
    in_maps = [{"enc_output": enc[i], "dec_output": dec[i]} for i in range(B)]
    res = run_bass_kernel_spmd(nc, in_maps, list(range(B)), trace=trace)
    out = np.stack([res.results[i]["out"] for i in range(B)], axis=0)
    return out, res


def kernel(enc_output, dec_output):
    out, _ = run(enc_output, dec_output)
    return out
